# revision 27
# baseline (speedup 1.0000x reference)
"""Trainium2 Bass kernel for nn_Castle_34351148433552 (sparse_attention).

Sharding: 8 cores = 2 batches x 4 head-groups. Core c handles batch c//4,
heads 4*(c%4) .. 4*(c%4)+3. W_qkv is sliced column-wise per head group (with
the q-scale, and the silu-via-tanh 0.5 factor on vu, folded in on the host),
W_out row-wise. Each core computes its partial output projection transposed
([1024, 2048]); the host sums the 4 partials per batch and transposes back.

Precision plan (tolerance 2e-2; this lands ~1.2e-2): matmul operands are
fp16 (1 cycle/row on the PE vs ~2.2 effective for fp32r HIGH) except (a)
the exp/AV path, bf16 for fp32-like range (exp has no max-subtraction, so
values reach ~e^30), and (b) LT/T1T, fp8e4m3 so the dominant n^3/6 Su
contraction runs as DoubleRow matmuls (two 128-row j-slabs per
instruction). PSUM accumulation is always fp32.

Device algorithm per (core, head), all in [d|c, n]-transposed layout:
  qkvT = (Wq_head^T x^T) via PE;  LT[j,k] = sigmoid(ku_j . qu_s_k) (j>k
  strict, affine_select fill 0 post-sigmoid; pair-interleaved fp8 layout,
  see _lt_offsets);  T1T[j,i] = vu_j . qc_s_i (i>=j, pre-halved, fp8,
  strips start at the pair-even diagonal corner);  SuT'[k,i] = sum_j
  LT*T1T via fp8 DoubleRow over j-block pairs, columns narrowed to the
  causal triangle;  silu(Su) = Su'*(1+tanh(Su')) (tanh shares the exp ACT
  table-set);  scoresT = ScT - silu, causal fill -1e30;  expT (bf16);
  AV via [vc|1] lhsT gives unnormalized out^T plus the softmax denominator
  in one PSUM accumulation;  normalize: PE-transpose the denominator row
  to [128,4] so the exact DVE reciprocal runs 128 lanes wide (the custom
  approx-reciprocal DVE ops miscompute on hardware), transpose back and
  broadcast via a rank-1 matmul;  project through W_out rows.
vc in natural layout comes from a separate x-stationary matmul pass over
all 4 heads at once (B2), not from PE transposes.  x stays resident in
SBUF (8 tiles) and feeds every phase.
"""

import os
import sys

import numpy as np

for _p in ("/opt/trn_rl_repo", os.path.expanduser("~/.axon_site/_ro/trn_rl_repo")):
    if os.path.isdir(_p) and _p not in sys.path:
        sys.path.insert(0, _p)
        break

H, D, NTOK, DIM = 16, 64, 2048, 1024
P = 128
NB = NTOK // P  # 16 row blocks
GW = 512        # i/k group width
NG = NTOK // GW  # 4 groups
HPC = 4         # heads per core
NCORES = 8
WHEAD = 6 * D   # 384 qkv columns per head
VC4W = HPC * D  # 256 vc columns for the whole head group


def _lt_offsets():
    """Pair-interleaved LT layout for fp8 DoubleRow Su matmuls.

    Strips (J, kg) for the J-pair (2Jp, 2Jp+1) are stored adjacently with a
    COMMON width wh (the odd strip's width; the even diagonal strip is padded
    by 128 columns that the strict j>k mask zeroes).  The Su lhsT is then the
    3D AP [128, 2, 128] sliced from the pair block.  Returns
    off[(J, kg)] = (col offset, stored width wh) and the total width.
    """
    off = {}
    o = 0
    for Jp in range(NB // 2):
        J0, J1 = 2 * Jp, 2 * Jp + 1
        for kg in range(J0 // 4 + 1):
            wh = GW if kg < J0 // 4 else (J1 % 4 + 1) * P
            off[(J0, kg)] = (o, wh)
            off[(J1, kg)] = (o + wh, wh)
            o += 2 * wh
    return off, o


_NC_CACHE = None


def build_nc():
    global _NC_CACHE
    if _NC_CACHE is not None:
        return _NC_CACHE

    import concourse.mybir as mybir
    import concourse.tile as tile
    from concourse import bacc

    dt = mybir.dt
    F32 = dt.float32
    F16 = dt.float16
    BF16 = dt.bfloat16
    F8 = dt.float8e4
    DR = mybir.MatmulPerfMode.DoubleRow
    AF = mybir.ActivationFunctionType
    ALU = mybir.AluOpType

    from concourse.masks import make_identity

    nc = bacc.Bacc(None, target_bir_lowering=False, debug=False)
    xT_d = nc.dram_tensor("xT", [DIM, NTOK], F16, kind="ExternalInput")
    wq_d = nc.dram_tensor("wq", [DIM, HPC * WHEAD + VC4W], F16,
                          kind="ExternalInput")
    wo_d = nc.dram_tensor("wo", [HPC * D, DIM], F16, kind="ExternalInput")
    out_d = nc.dram_tensor("outT", [DIM, NTOK], F32, kind="ExternalOutput")

    lt_off, LTW = _lt_offsets()

    phases = int(os.environ.get("KERNEL_PHASES", "6"))
    with tile.TileContext(nc) as tc:
        with (
            tc.tile_pool(name="const", bufs=1) as constp,
            tc.tile_pool(name="res", bufs=1) as resp,
            tc.tile_pool(name="qk2", bufs=3) as qk2p,
            tc.tile_pool(name="lt2", bufs=2) as lt2p,
            tc.tile_pool(name="work", bufs=4) as workp,
            tc.tile_pool(name="outsb", bufs=2) as outsbp,
            tc.tile_pool(name="ps", bufs=7, space="PSUM") as psp,
            tc.tile_pool(name="pavp", bufs=1, space="PSUM") as pavp,
        ):
            # ---------- constants / resident inputs ----------
            ones4 = constp.tile([4, D], BF16, tag="ones")
            nc.gpsimd.memset(ones4, 1.0)
            id128 = constp.tile([P, P], F32, tag="id128")
            make_identity(nc, id128)
            # [vc | 1] stationary blocks for all 4 heads: per head 16 slots
            # of [128, 65]; memset 1.0 first, vc parts overwritten by B2.
            vc4 = constp.tile([P, HPC * NB * (D + 1)], BF16, tag="vc4")
            nc.gpsimd.memset(vc4, 1.0)
            xres = [constp.tile([P, NTOK], F16, tag=f"x{k}", name=f"x{k}")
                    for k in range(8)]
            for k in range(8):
                nc.sync.dma_start(xres[k], xT_d[k * P:(k + 1) * P, :])
            wqvc = constp.tile([P, 8 * VC4W], F16, tag="wqvc")
            for k in range(8):
                nc.sync.dma_start(
                    wqvc[:, k * VC4W:(k + 1) * VC4W],
                    wq_d[k * P:(k + 1) * P, HPC * WHEAD:HPC * WHEAD + VC4W])
            wores = [resp.tile([P, DIM], F16, tag=f"wores{i}", name=f"wores{i}")
                     for i in range(2)]
            for i in range(2):
                nc.sync.dma_start(wores[i], wo_d[i * P:(i + 1) * P, :])

            # ---------- shared per-head tiles ----------
            attn = [resp.tile([P, NTOK], F16, tag=f"attn{i}", name=f"attn{i}")
                    for i in range(2)]

            # ---------- B2: vc natural layout for all 4 heads ----------
            vc4v = vc4.rearrange("p (h k c) -> p h k c", h=HPC, c=D + 1)
            for kb in range(NB):
                psv = psp.tile([P, VC4W], F32, tag="ps", name="psv")
                for kc in range(8):
                    nc.tensor.matmul(
                        psv,
                        xres[kc][:, kb * P:(kb + 1) * P],
                        wqvc[:, kc * VC4W:(kc + 1) * VC4W],
                        start=(kc == 0), stop=(kc == 7),
                    )
                nc.vector.tensor_copy(
                    vc4v[:, :, kb, 0:D],
                    psv.rearrange("p (h c) -> p h c", h=HPC))

            for hh in range(HPC):
                # ---------- A: qkv projection for this head ----------
                wqres = qk2p.tile([P, 8 * WHEAD], F16, tag="wqres",
                                  name="wqres")
                for kc in range(8):
                    nc.sync.dma_start(
                        wqres[:, kc * WHEAD:(kc + 1) * WHEAD],
                        wq_d[kc * P:(kc + 1) * P, hh * WHEAD:(hh + 1) * WHEAD],
                    )
                # t0=[qu|vu], t1=[ku|qc], t2=[vc|kc] (lhsT/rhs base
                # partitions must match)
                qkvT = [qk2p.tile([P, NTOK], F16, tag=f"qkvT{i}",
                                  name=f"qkvT{i}") for i in range(3)]
                # double-buffered across heads so C(h+1) overlaps D(h)
                LT = lt2p.tile([P, LTW], F8, tag="LT", name=f"LT{hh}")
                T1T = lt2p.tile([P, NB * GW], F8, tag="T1T",
                                name=f"T1T{hh}")
                for ng in range(NG):
                    ps = []
                    for ct in range(3):
                        ps.append(psp.tile([P, GW], F32, tag="ps",
                                           name=f"psq{ct}"))
                    for kc in range(8):
                        for ct in range(3):
                            nc.tensor.matmul(
                                ps[ct],
                                wqres[:, kc * WHEAD + ct * P:
                                      kc * WHEAD + (ct + 1) * P],
                                xres[kc][:, ng * GW:(ng + 1) * GW],
                                start=(kc == 0), stop=(kc == 7),
                            )
                    for ct in range(3):
                        nc.scalar.copy(
                            qkvT[ct][:, ng * GW:(ng + 1) * GW], ps[ct])

                # ---------- C: LT = sigmoid(ku . qu_s), strict j>k ----------
                # J-ascending so Su of early i-groups can start early
                for J in range(NB):
                    for kg in range(J // 4 + 1):
                        off, w = lt_off[(J, kg)]
                        pl = psp.tile([P, GW], F32, tag="ps", name="pl")
                        nc.tensor.matmul(
                            pl[:, :w],
                            qkvT[1][0:D, J * P:(J + 1) * P],
                            qkvT[0][0:D, kg * GW: kg * GW + w],
                            start=True, stop=True,
                        )
                        nc.scalar.activation(
                            LT[:, off:off + w], pl[:, :w], AF.Sigmoid)
                        if kg == J // 4:
                            # keep j > k: J*128 + p - kg*512 - f > 0
                            # (zeroes the even-strip pad columns too)
                            nc.gpsimd.affine_select(
                                out=LT[:, off:off + w], in_=LT[:, off:off + w],
                                compare_op=ALU.is_gt, fill=0.0,
                                base=J * P - kg * GW, pattern=[[-1, w]],
                                channel_multiplier=1,
                            )

                # ---------- D: attention, per 512-wide i-group ----------
                for ig in range(NG if phases >= 2 else 0):
                    nblk = 4 * ig + 4

                    # T1T strips (term1 transposed, pre-halved); columns
                    # before the block-diagonal corner are skipped entirely
                    for J in range(nblk):
                        # both halves of a J-pair start at the even strip's
                        # diagonal corner so the DoubleRow rhs is uniform
                        s = max(0, (J - J % 2) - 4 * ig) * P
                        w = GW - s
                        pt2 = psp.tile([P, GW], F32, tag="ps", name="pt2")
                        nc.tensor.matmul(
                            pt2[:, :w],
                            qkvT[0][D:2 * D, J * P:(J + 1) * P],
                            qkvT[1][D:2 * D, ig * GW + s:(ig + 1) * GW],
                            start=True, stop=True,
                        )
                        dst = T1T[:, J * GW + s:(J + 1) * GW]
                        nc.vector.tensor_copy(dst, pt2[:, :w])
                        if J >= 4 * ig:
                            # keep i >= j: (s - (J-4ig)*128) + f - p >= 0
                            nc.gpsimd.affine_select(
                                out=dst, in_=dst,
                                compare_op=ALU.is_ge, fill=0.0,
                                base=s - (J - 4 * ig) * P,
                                pattern=[[1, w]],
                                channel_multiplier=-1,
                            )

                    if phases < 3:
                        continue
                    # merged scores pass per k-block: Su' accumulate, Sc,
                    # silu-via-tanh, subtract, causal fill, exp, AV accum
                    pav_t = pavp.tile([D + 1, GW], F32, tag="av", name="pav")
                    for K in range(nblk):
                        sK = max(0, K - 4 * ig) * P
                        psu = psp.tile([P, GW], F32, tag="ps", name="psu")
                        Jp0, nJp = K // 2, nblk // 2
                        for Jp in range(Jp0, nJp):
                            sp = max(0, 2 * Jp - 4 * ig) * P
                            o_, wh = lt_off[(2 * Jp, K // 4)]
                            ltv = LT[:, o_: o_ + 2 * wh].rearrange(
                                "p (two f) -> p two f", two=2)
                            t1v = T1T[:, 2 * Jp * GW: (2 * Jp + 2) * GW
                                      ].rearrange("p (two f) -> p two f",
                                                  two=2)
                            nc.tensor.matmul(
                                psu[:, sp:],
                                ltv[:, :, (K % 4) * P:(K % 4 + 1) * P],
                                t1v[:, :, sp:],
                                start=(Jp == Jp0), stop=(Jp == nJp - 1),
                                perf_mode=DR,
                            )
                        psc = psp.tile([P, GW], F32, tag="ps", name="psc")
                        nc.tensor.matmul(
                            psc[:, sK:],
                            qkvT[2][D:2 * D, K * P:(K + 1) * P],
                            qkvT[1][D:2 * D, ig * GW + sK:(ig + 1) * GW],
                            start=True, stop=True,
                        )
                        if phases < 4:
                            continue
                        tnh = workp.tile([P, GW], F32, tag="tanh", name="tnh")
                        nc.scalar.activation(tnh[:, sK:], psu[:, sK:],
                                             AF.Tanh)
                        # silu(Su) in place: tnh = (tnh + 1) * Su'
                        nc.vector.scalar_tensor_tensor(
                            out=tnh[:, sK:], in0=tnh[:, sK:], scalar=1.0,
                            in1=psu[:, sK:],
                            op0=ALU.add, op1=ALU.mult,
                        )
                        sct = workp.tile([P, GW], F32, tag="sct", name="sct")
                        nc.vector.tensor_tensor(sct[:, sK:], psc[:, sK:],
                                                tnh[:, sK:],
                                                op=ALU.subtract)
                        if K >= 4 * ig:
                            # keep i >= k: local f - p >= 0
                            nc.gpsimd.affine_select(
                                out=sct[:, sK:], in_=sct[:, sK:],
                                compare_op=ALU.is_ge, fill=-1e30,
                                base=0, pattern=[[1, GW - sK]],
                                channel_multiplier=-1,
                            )
                        ext = workp.tile([P, GW], BF16, tag="ext", name="ext")
                        nc.scalar.activation(ext[:, sK:], sct[:, sK:], AF.Exp)
                        nc.tensor.matmul(
                            pav_t[:, sK:],
                            vc4[:, (hh * NB + K) * (D + 1):
                                (hh * NB + K + 1) * (D + 1)],
                            ext[:, sK:],
                            start=(K == 0), stop=(K == nblk - 1),
                        )

                    if phases < 5:
                        continue
                    # stage AV out of PSUM immediately (frees the bank)
                    avs = workp.tile([D + 1, GW], F32, tag="avs", name="avs")
                    nc.vector.tensor_copy(avs, pav_t)
                    # denominators sit on partition 64; PE-transpose them to
                    # [128, 4] so the exact DVE reciprocal runs 128-wide
                    # (the custom approx ops miscompute on hardware)
                    psD = psp.tile([P, 4], F32, tag="ps", name="psD")
                    for c_ in range(4):
                        nc.tensor.transpose(
                            psD[:, c_:c_ + 1],
                            avs[D:D + 1, c_ * P:(c_ + 1) * P],
                            id128[D:D + 1, D:D + 1])
                    rsb = workp.tile([P, 4], F32, tag="rsb", name="rsb")
                    nc.vector.reciprocal(rsb, psD)
                    psR = psp.tile([1, GW], F32, tag="ps", name="psR")
                    for c_ in range(4):
                        nc.tensor.transpose(
                            psR[0:1, c_ * P:(c_ + 1) * P],
                            rsb[:, c_:c_ + 1], id128)
                    recb = workp.tile([1, GW], BF16, tag="recb", name="recb")
                    nc.vector.tensor_copy(recb, psR)
                    pbc = psp.tile([D, GW], F32, tag="ps", name="pbc")
                    nc.tensor.matmul(pbc, ones4[0:1, :], recb,
                                     start=True, stop=True)
                    at = attn[hh // 2][(hh % 2) * D:(hh % 2 + 1) * D,
                                       ig * GW:(ig + 1) * GW]
                    nc.vector.tensor_tensor(at, avs[0:D, :], pbc,
                                            op=ALU.mult)

            # ---------- E: output projection (transposed) ----------
            # head pairs are stacked along partitions in attn[i]/wores[i],
            # so the c-contraction is two full-128-partition matmuls
            for dt_ in range(8 if phases >= 6 else 0):
                for ng in range(NG):
                    pp = psp.tile([P, GW], F32, tag="ps", name="pp")
                    for i in range(2):
                        nc.tensor.matmul(
                            pp, wores[i][:, dt_ * P:(dt_ + 1) * P],
                            attn[i][:, ng * GW:(ng + 1) * GW],
                            start=(i == 0), stop=(i == 1))
                    ot = outsbp.tile([P, GW], F32, tag="ot", name="ot")
                    if (dt_ * NG + ng) % 2 == 0:
                        nc.scalar.copy(ot, pp)
                    else:
                        nc.vector.tensor_copy(ot, pp)
                    nc.sync.dma_start(
                        out_d[dt_ * P:(dt_ + 1) * P, ng * GW:(ng + 1) * GW],
                        ot)

    nc.compile()
    _NC_CACHE = nc
    return nc


def shard_inputs(x, W_qkv, W_out):
    """Host-side sharding: per-core input dicts (fp16)."""
    x = np.asarray(x, np.float32)
    W_qkv = np.asarray(W_qkv, np.float32)
    W_out = np.asarray(W_out, np.float32)
    scale = D ** -0.5
    W6 = W_qkv.reshape(DIM, 6, H, D)
    in_maps = []
    xT = [np.ascontiguousarray(x[b].T).astype(np.float16) for b in range(2)]
    for c in range(NCORES):
        b, h0 = c // 4, 4 * (c % 4)
        Wc = W6[:, :, h0:h0 + HPC, :].transpose(0, 2, 1, 3).copy()  # [DIM,4,6,D]
        Wc[:, :, 0, :] *= scale  # qu
        Wc[:, :, 3, :] *= scale  # qc
        Wc[:, :, 2, :] *= 0.5    # vu -> Su' = Su/2 for silu-via-tanh
        vc4 = Wc[:, :, 5, :].reshape(DIM, HPC * D)  # vc, natural-layout pass
        # device c-order per head: [qu, vu, ku, qc, vc, kc]
        Wc = Wc[:, :, [0, 2, 1, 3, 5, 4], :]
        wq_c = np.concatenate(
            [Wc.reshape(DIM, HPC * WHEAD), vc4], axis=1).astype(np.float16)
        wo_c = np.ascontiguousarray(
            W_out.reshape(H, D, DIM)[h0:h0 + HPC].reshape(HPC * D, DIM)
        ).astype(np.float16)
        in_maps.append({
            "xT": xT[c // 4],
            "wq": np.ascontiguousarray(wq_c),
            "wo": wo_c,
        })
    return in_maps


def unshard_output(results):
    """results: list of 8 dicts with 'outT' [1024, 2048] partials."""
    outs = []
    for b in range(2):
        acc = results[4 * b]["outT"].astype(np.float32).copy()
        for c in range(4 * b + 1, 4 * b + 4):
            acc += results[c]["outT"]
        outs.append(acc.T)
    return np.stack(outs).astype(np.float32)


def kernel(x, W_qkv, W_out):
    from concourse.bass_utils import run_bass_kernel_spmd

    in_maps = shard_inputs(x, W_qkv, W_out)
    nc = build_nc()
    res = run_bass_kernel_spmd(nc, in_maps, core_ids=list(range(NCORES)))
    return unshard_output(res.results)


# revision 28
# speedup vs baseline: 1.1790x; 1.1790x over previous
"""Trainium2 Bass kernel for nn_Castle_34351148433552 (sparse_attention).

Sharding: 8 cores = 2 batches x 4 head-groups. Core c handles batch c//4,
heads 4*(c%4) .. 4*(c%4)+3. W_qkv is sliced column-wise per head group (with
the q-scale, and the silu-via-tanh 0.5 factor on vu, folded in on the host),
W_out row-wise. Each core computes its partial output projection transposed
([1024, 2048]); the host sums the 4 partials per batch and transposes back.

Precision plan (tolerance 2e-2; this lands ~1.2e-2): matmul operands are
fp16 (1 cycle/row on the PE vs ~2.2 effective for fp32r HIGH) except (a)
the exp/AV path, bf16 for fp32-like range (exp has no max-subtraction, so
values reach ~e^30), and (b) LT/T1T, fp8e4m3 so the dominant n^3/6 Su
contraction runs as DoubleRow matmuls (two 128-row j-slabs per
instruction). PSUM accumulation is always fp32.

Device algorithm per (core, head), all in [d|c, n]-transposed layout:
  qkvT = (Wq_head^T x^T) via PE;  LT[j,k] = sigmoid(ku_j . qu_s_k) (j>k
  strict, affine_select fill 0 post-sigmoid; pair-interleaved fp8 layout,
  see _lt_offsets);  T1T[j,i] = vu_j . qc_s_i (i>=j, pre-halved, fp8,
  strips start at the pair-even diagonal corner);  SuT'[k,i] = sum_j
  LT*T1T via fp8 DoubleRow over j-block pairs, columns narrowed to the
  causal triangle;  silu(Su) = Su'*(1+tanh(Su')) (tanh shares the exp ACT
  table-set);  scoresT = ScT - silu, causal fill -1e30;  expT (bf16);
  AV via [vc|1] lhsT gives unnormalized out^T plus the softmax denominator
  in one PSUM accumulation;  normalize: PE-transpose the denominator row
  to [128,4] so the exact DVE reciprocal runs 128 lanes wide (the custom
  approx-reciprocal DVE ops miscompute on hardware), transpose back and
  broadcast via a rank-1 matmul;  project through W_out rows.
vc in natural layout comes from a separate x-stationary matmul pass over
all 4 heads at once (B2), not from PE transposes.  x stays resident in
SBUF (8 tiles) and feeds every phase.
"""

import os
import sys

import numpy as np

for _p in ("/opt/trn_rl_repo", os.path.expanduser("~/.axon_site/_ro/trn_rl_repo")):
    if os.path.isdir(_p) and _p not in sys.path:
        sys.path.insert(0, _p)
        break

H, D, NTOK, DIM = 16, 64, 2048, 1024
P = 128
NB = NTOK // P  # 16 row blocks
GW = 512        # i/k group width
NG = NTOK // GW  # 4 groups
HPC = 4         # heads per core
NCORES = 8
WHEAD = 6 * D   # 384 qkv columns per head
VC4W = HPC * D  # 256 vc columns for the whole head group


def _lt_offsets():
    """Pair-interleaved LT layout for fp8 DoubleRow Su matmuls.

    Strips (J, kg) for the J-pair (2Jp, 2Jp+1) are stored adjacently with a
    COMMON width wh (the odd strip's width; the even diagonal strip is padded
    by 128 columns that the strict j>k mask zeroes).  The Su lhsT is then the
    3D AP [128, 2, 128] sliced from the pair block.  Returns
    off[(J, kg)] = (col offset, stored width wh) and the total width.
    """
    off = {}
    o = 0
    for Jp in range(NB // 2):
        J0, J1 = 2 * Jp, 2 * Jp + 1
        for kg in range(J0 // 4 + 1):
            wh = GW if kg < J0 // 4 else (J1 % 4 + 1) * P
            off[(J0, kg)] = (o, wh)
            off[(J1, kg)] = (o + wh, wh)
            o += 2 * wh
    return off, o


_NC_CACHE = None


def build_nc():
    global _NC_CACHE
    if _NC_CACHE is not None:
        return _NC_CACHE

    import concourse.mybir as mybir
    import concourse.tile as tile
    from concourse import bacc

    dt = mybir.dt
    F32 = dt.float32
    F16 = dt.float16
    BF16 = dt.bfloat16
    F8 = dt.float8e4
    DR = mybir.MatmulPerfMode.DoubleRow
    AF = mybir.ActivationFunctionType
    ALU = mybir.AluOpType

    from concourse.masks import make_identity

    nc = bacc.Bacc(None, target_bir_lowering=False, debug=False)
    xT_d = nc.dram_tensor("xT", [DIM, NTOK], F16, kind="ExternalInput")
    wq_d = nc.dram_tensor("wq", [DIM, HPC * WHEAD + VC4W], F16,
                          kind="ExternalInput")
    wo_d = nc.dram_tensor("wo", [HPC * D, DIM], F16, kind="ExternalInput")
    out_d = nc.dram_tensor("outT", [DIM, NTOK], F32, kind="ExternalOutput")

    lt_off, LTW = _lt_offsets()

    phases = int(os.environ.get("KERNEL_PHASES", "6"))
    with tile.TileContext(nc) as tc:
        with (
            tc.tile_pool(name="const", bufs=1) as constp,
            tc.tile_pool(name="res", bufs=1) as resp,
            tc.tile_pool(name="qk2", bufs=3) as qk2p,
            tc.tile_pool(name="work", bufs=4) as workp,
            tc.tile_pool(name="outsb", bufs=2) as outsbp,
            tc.tile_pool(name="ps", bufs=7, space="PSUM") as psp,
            tc.tile_pool(name="pavp", bufs=1, space="PSUM") as pavp,
        ):
            # ---------- constants / resident inputs ----------
            ones4 = constp.tile([4, D], BF16, tag="ones")
            nc.gpsimd.memset(ones4, 1.0)
            id128 = constp.tile([P, P], F32, tag="id128")
            make_identity(nc, id128)
            # [vc | 1] stationary blocks for all 4 heads: per head 16 slots
            # of [128, 65]; memset 1.0 first, vc parts overwritten by B2.
            vc4 = constp.tile([P, HPC * NB * (D + 1)], BF16, tag="vc4")
            nc.gpsimd.memset(vc4, 1.0)
            xres = [constp.tile([P, NTOK], F16, tag=f"x{k}", name=f"x{k}")
                    for k in range(8)]
            for k in range(8):
                nc.sync.dma_start(xres[k], xT_d[k * P:(k + 1) * P, :])
            wqvc = constp.tile([P, 8 * VC4W], F16, tag="wqvc")
            for k in range(8):
                nc.sync.dma_start(
                    wqvc[:, k * VC4W:(k + 1) * VC4W],
                    wq_d[k * P:(k + 1) * P, HPC * WHEAD:HPC * WHEAD + VC4W])
            wores = [resp.tile([P, DIM], F16, tag=f"wores{i}", name=f"wores{i}")
                     for i in range(2)]
            for i in range(2):
                nc.sync.dma_start(wores[i], wo_d[i * P:(i + 1) * P, :])

            # ---------- shared per-head tiles ----------
            LT = resp.tile([P, LTW], F8, tag="LT")
            T1T = resp.tile([P, NB * GW], F8, tag="T1T")
            attn = [resp.tile([P, NTOK], F16, tag=f"attn{i}", name=f"attn{i}")
                    for i in range(2)]

            # ---------- B2: vc natural layout for all 4 heads ----------
            vc4v = vc4.rearrange("p (h k c) -> p h k c", h=HPC, c=D + 1)
            for kb in range(NB):
                psv = psp.tile([P, VC4W], F32, tag="ps", name="psv")
                for kc in range(8):
                    nc.tensor.matmul(
                        psv,
                        xres[kc][:, kb * P:(kb + 1) * P],
                        wqvc[:, kc * VC4W:(kc + 1) * VC4W],
                        start=(kc == 0), stop=(kc == 7),
                    )
                nc.vector.tensor_copy(
                    vc4v[:, :, kb, 0:D],
                    psv.rearrange("p (h c) -> p h c", h=HPC))

            for hh in range(HPC):
                # ---------- A: qkv projection for this head ----------
                wqres = qk2p.tile([P, 8 * WHEAD], F16, tag="wqres",
                                  name="wqres")
                for kc in range(8):
                    nc.sync.dma_start(
                        wqres[:, kc * WHEAD:(kc + 1) * WHEAD],
                        wq_d[kc * P:(kc + 1) * P, hh * WHEAD:(hh + 1) * WHEAD],
                    )
                # t0=[qu|vu], t1=[ku|qc], t2=[vc|kc] (lhsT/rhs base
                # partitions must match)
                qkvT = [qk2p.tile([P, NTOK], F16, tag=f"qkvT{i}",
                                  name=f"qkvT{i}") for i in range(3)]
                for ng in range(NG):
                    ps = []
                    for ct in range(3):
                        ps.append(psp.tile([P, GW], F32, tag="ps",
                                           name=f"psq{ct}"))
                    for kc in range(8):
                        for ct in range(3):
                            nc.tensor.matmul(
                                ps[ct],
                                wqres[:, kc * WHEAD + ct * P:
                                      kc * WHEAD + (ct + 1) * P],
                                xres[kc][:, ng * GW:(ng + 1) * GW],
                                start=(kc == 0), stop=(kc == 7),
                            )
                    for ct in range(3):
                        nc.scalar.copy(
                            qkvT[ct][:, ng * GW:(ng + 1) * GW], ps[ct])

                # ---------- C: LT = sigmoid(ku . qu_s), strict j>k ----------
                # J-ascending so Su of early i-groups can start early
                for J in range(NB):
                    for kg in range(J // 4 + 1):
                        off, w = lt_off[(J, kg)]
                        pl = psp.tile([P, GW], F32, tag="ps", name="pl")
                        nc.tensor.matmul(
                            pl[:, :w],
                            qkvT[1][0:D, J * P:(J + 1) * P],
                            qkvT[0][0:D, kg * GW: kg * GW + w],
                            start=True, stop=True,
                        )
                        nc.scalar.activation(
                            LT[:, off:off + w], pl[:, :w], AF.Sigmoid)
                        if kg == J // 4:
                            # keep j > k: J*128 + p - kg*512 - f > 0
                            # (zeroes the even-strip pad columns too)
                            nc.gpsimd.affine_select(
                                out=LT[:, off:off + w], in_=LT[:, off:off + w],
                                compare_op=ALU.is_gt, fill=0.0,
                                base=J * P - kg * GW, pattern=[[-1, w]],
                                channel_multiplier=1,
                            )

                # ---------- D: attention, per 512-wide i-group ----------
                for ig in range(NG if phases >= 2 else 0):
                    nblk = 4 * ig + 4

                    # T1T strips (term1 transposed, pre-halved); columns
                    # before the block-diagonal corner are skipped entirely
                    for J in range(nblk):
                        # both halves of a J-pair start at the even strip's
                        # diagonal corner so the DoubleRow rhs is uniform
                        s = max(0, (J - J % 2) - 4 * ig) * P
                        w = GW - s
                        pt2 = psp.tile([P, GW], F32, tag="ps", name="pt2")
                        nc.tensor.matmul(
                            pt2[:, :w],
                            qkvT[0][D:2 * D, J * P:(J + 1) * P],
                            qkvT[1][D:2 * D, ig * GW + s:(ig + 1) * GW],
                            start=True, stop=True,
                        )
                        dst = T1T[:, J * GW + s:(J + 1) * GW]
                        nc.vector.tensor_copy(dst, pt2[:, :w])
                        if J >= 4 * ig:
                            # keep i >= j: (s - (J-4ig)*128) + f - p >= 0
                            nc.gpsimd.affine_select(
                                out=dst, in_=dst,
                                compare_op=ALU.is_ge, fill=0.0,
                                base=s - (J - 4 * ig) * P,
                                pattern=[[1, w]],
                                channel_multiplier=-1,
                            )

                    if phases < 3:
                        continue
                    # merged scores pass per k-block: Su' accumulate, Sc,
                    # silu-via-tanh, subtract, causal fill, exp, AV accum
                    pav_t = pavp.tile([D + 1, GW], F32, tag="av", name="pav")
                    for K in range(nblk):
                        sK = max(0, K - 4 * ig) * P
                        psu = psp.tile([P, GW], F32, tag="ps", name="psu")
                        Jp0, nJp = K // 2, nblk // 2
                        for Jp in range(Jp0, nJp):
                            sp = max(0, 2 * Jp - 4 * ig) * P
                            o_, wh = lt_off[(2 * Jp, K // 4)]
                            ltv = LT[:, o_: o_ + 2 * wh].rearrange(
                                "p (two f) -> p two f", two=2)
                            t1v = T1T[:, 2 * Jp * GW: (2 * Jp + 2) * GW
                                      ].rearrange("p (two f) -> p two f",
                                                  two=2)
                            nc.tensor.matmul(
                                psu[:, sp:],
                                ltv[:, :, (K % 4) * P:(K % 4 + 1) * P],
                                t1v[:, :, sp:],
                                start=(Jp == Jp0), stop=(Jp == nJp - 1),
                                perf_mode=DR,
                            )
                        psc = psp.tile([P, GW], F32, tag="ps", name="psc")
                        nc.tensor.matmul(
                            psc[:, sK:],
                            qkvT[2][D:2 * D, K * P:(K + 1) * P],
                            qkvT[1][D:2 * D, ig * GW + sK:(ig + 1) * GW],
                            start=True, stop=True,
                        )
                        if phases < 4:
                            continue
                        tnh = workp.tile([P, GW], F32, tag="tanh", name="tnh")
                        nc.scalar.activation(tnh[:, sK:], psu[:, sK:],
                                             AF.Tanh)
                        # silu(Su) in place: tnh = (tnh + 1) * Su'
                        nc.vector.scalar_tensor_tensor(
                            out=tnh[:, sK:], in0=tnh[:, sK:], scalar=1.0,
                            in1=psu[:, sK:],
                            op0=ALU.add, op1=ALU.mult,
                        )
                        sct = workp.tile([P, GW], F32, tag="sct", name="sct")
                        nc.vector.tensor_tensor(sct[:, sK:], psc[:, sK:],
                                                tnh[:, sK:],
                                                op=ALU.subtract)
                        if K >= 4 * ig:
                            # keep i >= k: local f - p >= 0
                            nc.gpsimd.affine_select(
                                out=sct[:, sK:], in_=sct[:, sK:],
                                compare_op=ALU.is_ge, fill=-1e30,
                                base=0, pattern=[[1, GW - sK]],
                                channel_multiplier=-1,
                            )
                        ext = workp.tile([P, GW], BF16, tag="ext", name="ext")
                        nc.scalar.activation(ext[:, sK:], sct[:, sK:], AF.Exp)
                        nc.tensor.matmul(
                            pav_t[:, sK:],
                            vc4[:, (hh * NB + K) * (D + 1):
                                (hh * NB + K + 1) * (D + 1)],
                            ext[:, sK:],
                            start=(K == 0), stop=(K == nblk - 1),
                        )

                    if phases < 5:
                        continue
                    # stage AV out of PSUM immediately (frees the bank)
                    avs = workp.tile([D + 1, GW], F32, tag="avs", name="avs")
                    nc.vector.tensor_copy(avs, pav_t)
                    # denominators sit on partition 64; PE-transpose them to
                    # [128, 4] so the exact DVE reciprocal runs 128-wide
                    # (the custom approx ops miscompute on hardware)
                    psD = psp.tile([P, 4], F32, tag="ps", name="psD")
                    for c_ in range(4):
                        nc.tensor.transpose(
                            psD[:, c_:c_ + 1],
                            avs[D:D + 1, c_ * P:(c_ + 1) * P],
                            id128[D:D + 1, D:D + 1])
                    rsb = workp.tile([P, 4], F32, tag="rsb", name="rsb")
                    nc.vector.reciprocal(rsb, psD)
                    psR = psp.tile([1, GW], F32, tag="ps", name="psR")
                    for c_ in range(4):
                        nc.tensor.transpose(
                            psR[0:1, c_ * P:(c_ + 1) * P],
                            rsb[:, c_:c_ + 1], id128)
                    recb = workp.tile([1, GW], BF16, tag="recb", name="recb")
                    nc.vector.tensor_copy(recb, psR)
                    pbc = psp.tile([D, GW], F32, tag="ps", name="pbc")
                    nc.tensor.matmul(pbc, ones4[0:1, :], recb,
                                     start=True, stop=True)
                    at = attn[hh // 2][(hh % 2) * D:(hh % 2 + 1) * D,
                                       ig * GW:(ig + 1) * GW]
                    nc.vector.tensor_tensor(at, avs[0:D, :], pbc,
                                            op=ALU.mult)

            # ---------- E: output projection (transposed) ----------
            # head pairs are stacked along partitions in attn[i]/wores[i],
            # so the c-contraction is two full-128-partition matmuls
            for dt_ in range(8 if phases >= 6 else 0):
                for ng in range(NG):
                    pp = psp.tile([P, GW], F32, tag="ps", name="pp")
                    for i in range(2):
                        nc.tensor.matmul(
                            pp, wores[i][:, dt_ * P:(dt_ + 1) * P],
                            attn[i][:, ng * GW:(ng + 1) * GW],
                            start=(i == 0), stop=(i == 1))
                    ot = outsbp.tile([P, GW], F32, tag="ot", name="ot")
                    if (dt_ * NG + ng) % 2 == 0:
                        nc.scalar.copy(ot, pp)
                    else:
                        nc.vector.tensor_copy(ot, pp)
                    nc.sync.dma_start(
                        out_d[dt_ * P:(dt_ + 1) * P, ng * GW:(ng + 1) * GW],
                        ot)

    nc.compile()
    _NC_CACHE = nc
    return nc


def shard_inputs(x, W_qkv, W_out):
    """Host-side sharding: per-core input dicts (fp16)."""
    x = np.asarray(x, np.float32)
    W_qkv = np.asarray(W_qkv, np.float32)
    W_out = np.asarray(W_out, np.float32)
    scale = D ** -0.5
    W6 = W_qkv.reshape(DIM, 6, H, D)
    in_maps = []
    xT = [np.ascontiguousarray(x[b].T).astype(np.float16) for b in range(2)]
    for c in range(NCORES):
        b, h0 = c // 4, 4 * (c % 4)
        Wc = W6[:, :, h0:h0 + HPC, :].transpose(0, 2, 1, 3).copy()  # [DIM,4,6,D]
        Wc[:, :, 0, :] *= scale  # qu
        Wc[:, :, 3, :] *= scale  # qc
        Wc[:, :, 2, :] *= 0.5    # vu -> Su' = Su/2 for silu-via-tanh
        vc4 = Wc[:, :, 5, :].reshape(DIM, HPC * D)  # vc, natural-layout pass
        # device c-order per head: [qu, vu, ku, qc, vc, kc]
        Wc = Wc[:, :, [0, 2, 1, 3, 5, 4], :]
        wq_c = np.concatenate(
            [Wc.reshape(DIM, HPC * WHEAD), vc4], axis=1).astype(np.float16)
        wo_c = np.ascontiguousarray(
            W_out.reshape(H, D, DIM)[h0:h0 + HPC].reshape(HPC * D, DIM)
        ).astype(np.float16)
        in_maps.append({
            "xT": xT[c // 4],
            "wq": np.ascontiguousarray(wq_c),
            "wo": wo_c,
        })
    return in_maps


def unshard_output(results):
    """results: list of 8 dicts with 'outT' [1024, 2048] partials."""
    outs = []
    for b in range(2):
        acc = results[4 * b]["outT"].astype(np.float32).copy()
        for c in range(4 * b + 1, 4 * b + 4):
            acc += results[c]["outT"]
        outs.append(acc.T)
    return np.stack(outs).astype(np.float32)


def kernel(x, W_qkv, W_out):
    from concourse.bass_utils import run_bass_kernel_spmd

    in_maps = shard_inputs(x, W_qkv, W_out)
    nc = build_nc()
    res = run_bass_kernel_spmd(nc, in_maps, core_ids=list(range(NCORES)))
    return unshard_output(res.results)


# revision 29
# speedup vs baseline: 1.2324x; 1.0452x over previous
"""Trainium2 Bass kernel for nn_Castle_34351148433552 (sparse_attention).

Sharding: 8 cores = 2 batches x 4 head-groups. Core c handles batch c//4,
heads 4*(c%4) .. 4*(c%4)+3. W_qkv is sliced column-wise per head group (with
the q-scale, and the silu-via-tanh 0.5 factor on vu, folded in on the host),
W_out row-wise. Each core computes its partial output projection transposed
([1024, 2048]); the host sums the 4 partials per batch and transposes back.

Precision plan (tolerance 2e-2; this lands ~1.2e-2): matmul operands are
fp16 (1 cycle/row on the PE vs ~2.2 effective for fp32r HIGH) except (a)
the exp/AV path, bf16 for fp32-like range (exp has no max-subtraction, so
values reach ~e^30), and (b) LT/T1T, fp8e4m3 so the dominant n^3/6 Su
contraction runs as DoubleRow matmuls (two 128-row j-slabs per
instruction). PSUM accumulation is always fp32.

Device algorithm per (core, head), all in [d|c, n]-transposed layout:
  qkvT = (Wq_head^T x^T) via PE;  LT[j,k] = sigmoid(ku_j . qu_s_k) (j>k
  strict, affine_select fill 0 post-sigmoid; pair-interleaved fp8 layout,
  see _lt_offsets);  T1T[j,i] = vu_j . qc_s_i (i>=j, pre-halved, fp8,
  strips start at the pair-even diagonal corner);  SuT'[k,i] = sum_j
  LT*T1T via fp8 DoubleRow over j-block pairs, columns narrowed to the
  causal triangle;  silu(Su) = Su'*(1+tanh(Su')) (tanh shares the exp ACT
  table-set);  scoresT = ScT - silu, causal fill -1e30;  expT (bf16);
  AV via [vc|1] lhsT gives unnormalized out^T plus the softmax denominator
  in one PSUM accumulation;  normalize: PE-transpose the denominator row
  to [128,4] so the exact DVE reciprocal runs 128 lanes wide (the custom
  approx-reciprocal DVE ops miscompute on hardware), transpose back and
  broadcast via a rank-1 matmul;  project through W_out rows.
vc in natural layout comes from a separate x-stationary matmul pass over
all 4 heads at once (B2), not from PE transposes.  x stays resident in
SBUF (8 tiles) and feeds every phase.
"""

import os
import sys

import numpy as np

for _p in ("/opt/trn_rl_repo", os.path.expanduser("~/.axon_site/_ro/trn_rl_repo")):
    if os.path.isdir(_p) and _p not in sys.path:
        sys.path.insert(0, _p)
        break

H, D, NTOK, DIM = 16, 64, 2048, 1024
P = 128
NB = NTOK // P  # 16 row blocks
GW = 512        # i/k group width
NG = NTOK // GW  # 4 groups
HPC = 4         # heads per core
NCORES = 8
WHEAD = 6 * D   # 384 qkv columns per head
VC4W = HPC * D  # 256 vc columns for the whole head group


def _lt_offsets():
    """Pair-interleaved LT layout for fp8 DoubleRow Su matmuls.

    Strips (J, kg) for the J-pair (2Jp, 2Jp+1) are stored adjacently with a
    COMMON width wh (the odd strip's width; the even diagonal strip is padded
    by 128 columns that the strict j>k mask zeroes).  The Su lhsT is then the
    3D AP [128, 2, 128] sliced from the pair block.  Returns
    off[(J, kg)] = (col offset, stored width wh) and the total width.
    """
    off = {}
    o = 0
    for Jp in range(NB // 2):
        J0, J1 = 2 * Jp, 2 * Jp + 1
        for kg in range(J0 // 4 + 1):
            wh = GW if kg < J0 // 4 else (J1 % 4 + 1) * P
            off[(J0, kg)] = (o, wh)
            off[(J1, kg)] = (o + wh, wh)
            o += 2 * wh
    return off, o


_NC_CACHE = None


def build_nc():
    global _NC_CACHE
    if _NC_CACHE is not None:
        return _NC_CACHE

    import concourse.mybir as mybir
    import concourse.tile as tile
    from concourse import bacc

    dt = mybir.dt
    F32 = dt.float32
    F16 = dt.float16
    BF16 = dt.bfloat16
    F8 = dt.float8e4
    DR = mybir.MatmulPerfMode.DoubleRow
    AF = mybir.ActivationFunctionType
    ALU = mybir.AluOpType

    from concourse.masks import make_identity

    nc = bacc.Bacc(None, target_bir_lowering=False, debug=False)
    xT_d = nc.dram_tensor("xT", [DIM, NTOK], F16, kind="ExternalInput")
    wq_d = nc.dram_tensor("wq", [DIM, HPC * WHEAD + VC4W], F16,
                          kind="ExternalInput")
    wo_d = nc.dram_tensor("wo", [HPC * D, DIM], F16, kind="ExternalInput")
    out_d = nc.dram_tensor("outT", [DIM, NTOK], F32, kind="ExternalOutput")

    lt_off, LTW = _lt_offsets()

    phases = int(os.environ.get("KERNEL_PHASES", "6"))
    with tile.TileContext(nc) as tc:
        with (
            tc.tile_pool(name="const", bufs=1) as constp,
            tc.tile_pool(name="res", bufs=1) as resp,
            tc.tile_pool(name="qk2", bufs=3) as qk2p,
            tc.tile_pool(name="work", bufs=4) as workp,
            tc.tile_pool(name="outsb", bufs=4) as outsbp,
            tc.tile_pool(name="ps", bufs=7, space="PSUM") as psp,
            tc.tile_pool(name="pavp", bufs=1, space="PSUM") as pavp,
        ):
            # ---------- constants / resident inputs ----------
            ones4 = constp.tile([4, D], BF16, tag="ones")
            nc.gpsimd.memset(ones4, 1.0)
            id128 = constp.tile([P, P], F32, tag="id128")
            make_identity(nc, id128)
            # [vc | 1] stationary blocks for all 4 heads: per head 16 slots
            # of [128, 65]; memset 1.0 first, vc parts overwritten by B2.
            vc4 = constp.tile([P, HPC * NB * (D + 1)], BF16, tag="vc4")
            nc.gpsimd.memset(vc4, 1.0)
            xres = [constp.tile([P, NTOK], F16, tag=f"x{k}", name=f"x{k}")
                    for k in range(8)]
            for k in range(8):
                nc.sync.dma_start(xres[k], xT_d[k * P:(k + 1) * P, :])
            wqvc = constp.tile([P, 8 * VC4W], F16, tag="wqvc")
            for k in range(8):
                nc.sync.dma_start(
                    wqvc[:, k * VC4W:(k + 1) * VC4W],
                    wq_d[k * P:(k + 1) * P, HPC * WHEAD:HPC * WHEAD + VC4W])
            wores = [resp.tile([P, DIM], F16, tag=f"wores{i}", name=f"wores{i}")
                     for i in range(2)]
            for i in range(2):
                nc.sync.dma_start(wores[i], wo_d[i * P:(i + 1) * P, :])

            # ---------- shared per-head tiles ----------
            LT = resp.tile([P, LTW], F8, tag="LT")
            T1T = resp.tile([P, NB * GW], F8, tag="T1T")
            attn = [resp.tile([P, NTOK], F16, tag=f"attn{i}", name=f"attn{i}")
                    for i in range(2)]

            # ---------- B2: vc natural layout for all 4 heads ----------
            vc4v = vc4.rearrange("p (h k c) -> p h k c", h=HPC, c=D + 1)
            for kb in range(NB):
                psv = psp.tile([P, VC4W], F32, tag="ps", name="psv")
                for kc in range(8):
                    nc.tensor.matmul(
                        psv,
                        xres[kc][:, kb * P:(kb + 1) * P],
                        wqvc[:, kc * VC4W:(kc + 1) * VC4W],
                        start=(kc == 0), stop=(kc == 7),
                    )
                nc.vector.tensor_copy(
                    vc4v[:, :, kb, 0:D],
                    psv.rearrange("p (h c) -> p h c", h=HPC))

            for hh in range(HPC):
                # ---------- A: qkv projection for this head ----------
                wqres = qk2p.tile([P, 8 * WHEAD], F16, tag="wqres",
                                  name="wqres")
                for kc in range(8):
                    nc.sync.dma_start(
                        wqres[:, kc * WHEAD:(kc + 1) * WHEAD],
                        wq_d[kc * P:(kc + 1) * P, hh * WHEAD:(hh + 1) * WHEAD],
                    )
                # t0=[qu|vu], t1=[ku|qc], t2=[vc|kc] (lhsT/rhs base
                # partitions must match)
                qkvT = [qk2p.tile([P, NTOK], F16, tag=f"qkvT{i}",
                                  name=f"qkvT{i}") for i in range(3)]
                for ng in range(NG):
                    ps = []
                    for ct in range(3):
                        ps.append(psp.tile([P, GW], F32, tag="ps",
                                           name=f"psq{ct}"))
                    for kc in range(8):
                        for ct in range(3):
                            nc.tensor.matmul(
                                ps[ct],
                                wqres[:, kc * WHEAD + ct * P:
                                      kc * WHEAD + (ct + 1) * P],
                                xres[kc][:, ng * GW:(ng + 1) * GW],
                                start=(kc == 0), stop=(kc == 7),
                            )
                    for ct in range(3):
                        nc.scalar.copy(
                            qkvT[ct][:, ng * GW:(ng + 1) * GW], ps[ct])

                # ---------- C: LT = sigmoid(ku . qu_s), strict j>k ----------
                # J-ascending so Su of early i-groups can start early
                for J in range(NB):
                    for kg in range(J // 4 + 1):
                        off, w = lt_off[(J, kg)]
                        pl = psp.tile([P, GW], F32, tag="ps", name="pl")
                        nc.tensor.matmul(
                            pl[:, :w],
                            qkvT[1][0:D, J * P:(J + 1) * P],
                            qkvT[0][0:D, kg * GW: kg * GW + w],
                            start=True, stop=True,
                        )
                        nc.scalar.activation(
                            LT[:, off:off + w], pl[:, :w], AF.Sigmoid)
                        if kg == J // 4:
                            # keep j > k: J*128 + p - kg*512 - f > 0
                            # (zeroes the even-strip pad columns too)
                            nc.gpsimd.affine_select(
                                out=LT[:, off:off + w], in_=LT[:, off:off + w],
                                compare_op=ALU.is_gt, fill=0.0,
                                base=J * P - kg * GW, pattern=[[-1, w]],
                                channel_multiplier=1,
                            )

                # ---------- D: attention, per 512-wide i-group ----------
                for ig in range(NG if phases >= 2 else 0):
                    nblk = 4 * ig + 4

                    # T1T strips (term1 transposed, pre-halved); columns
                    # before the block-diagonal corner are skipped entirely
                    for J in range(nblk):
                        # both halves of a J-pair start at the even strip's
                        # diagonal corner so the DoubleRow rhs is uniform
                        s = max(0, (J - J % 2) - 4 * ig) * P
                        w = GW - s
                        pt2 = psp.tile([P, GW], F32, tag="ps", name="pt2")
                        nc.tensor.matmul(
                            pt2[:, :w],
                            qkvT[0][D:2 * D, J * P:(J + 1) * P],
                            qkvT[1][D:2 * D, ig * GW + s:(ig + 1) * GW],
                            start=True, stop=True,
                        )
                        dst = T1T[:, J * GW + s:(J + 1) * GW]
                        nc.vector.tensor_copy(dst, pt2[:, :w])
                        if J >= 4 * ig:
                            # keep i >= j: (s - (J-4ig)*128) + f - p >= 0
                            nc.gpsimd.affine_select(
                                out=dst, in_=dst,
                                compare_op=ALU.is_ge, fill=0.0,
                                base=s - (J - 4 * ig) * P,
                                pattern=[[1, w]],
                                channel_multiplier=-1,
                            )

                    if phases < 3:
                        continue
                    # merged scores pass per k-block: Su' accumulate, Sc,
                    # silu-via-tanh, subtract, causal fill, exp, AV accum
                    pav_t = pavp.tile([D + 1, GW], F32, tag="av", name="pav")
                    for K in range(nblk):
                        sK = max(0, K - 4 * ig) * P
                        psu = psp.tile([P, GW], F32, tag="ps", name="psu")
                        Jp0, nJp = K // 2, nblk // 2
                        for Jp in range(Jp0, nJp):
                            sp = max(0, 2 * Jp - 4 * ig) * P
                            o_, wh = lt_off[(2 * Jp, K // 4)]
                            ltv = LT[:, o_: o_ + 2 * wh].rearrange(
                                "p (two f) -> p two f", two=2)
                            t1v = T1T[:, 2 * Jp * GW: (2 * Jp + 2) * GW
                                      ].rearrange("p (two f) -> p two f",
                                                  two=2)
                            nc.tensor.matmul(
                                psu[:, sp:],
                                ltv[:, :, (K % 4) * P:(K % 4 + 1) * P],
                                t1v[:, :, sp:],
                                start=(Jp == Jp0), stop=(Jp == nJp - 1),
                                perf_mode=DR,
                            )
                        psc = psp.tile([P, GW], F32, tag="ps", name="psc")
                        nc.tensor.matmul(
                            psc[:, sK:],
                            qkvT[2][D:2 * D, K * P:(K + 1) * P],
                            qkvT[1][D:2 * D, ig * GW + sK:(ig + 1) * GW],
                            start=True, stop=True,
                        )
                        if phases < 4:
                            continue
                        tnh = workp.tile([P, GW], F32, tag="tanh", name="tnh")
                        nc.scalar.activation(tnh[:, sK:], psu[:, sK:],
                                             AF.Tanh)
                        # silu(Su) in place: tnh = (tnh + 1) * Su'
                        nc.vector.scalar_tensor_tensor(
                            out=tnh[:, sK:], in0=tnh[:, sK:], scalar=1.0,
                            in1=psu[:, sK:],
                            op0=ALU.add, op1=ALU.mult,
                        )
                        sct = workp.tile([P, GW], F32, tag="sct", name="sct")
                        nc.vector.tensor_tensor(sct[:, sK:], psc[:, sK:],
                                                tnh[:, sK:],
                                                op=ALU.subtract)
                        if K >= 4 * ig:
                            # keep i >= k: local f - p >= 0
                            nc.gpsimd.affine_select(
                                out=sct[:, sK:], in_=sct[:, sK:],
                                compare_op=ALU.is_ge, fill=-1e30,
                                base=0, pattern=[[1, GW - sK]],
                                channel_multiplier=-1,
                            )
                        ext = workp.tile([P, GW], BF16, tag="ext", name="ext")
                        nc.scalar.activation(ext[:, sK:], sct[:, sK:], AF.Exp)
                        nc.tensor.matmul(
                            pav_t[:, sK:],
                            vc4[:, (hh * NB + K) * (D + 1):
                                (hh * NB + K + 1) * (D + 1)],
                            ext[:, sK:],
                            start=(K == 0), stop=(K == nblk - 1),
                        )

                    if phases < 5:
                        continue
                    # stage AV out of PSUM immediately (frees the bank)
                    avs = workp.tile([D + 1, GW], F32, tag="avs", name="avs")
                    nc.vector.tensor_copy(avs, pav_t)
                    # denominators sit on partition 64; PE-transpose them to
                    # [128, 4] so the exact DVE reciprocal runs 128-wide
                    # (the custom approx ops miscompute on hardware)
                    psD = psp.tile([P, 4], F32, tag="ps", name="psD")
                    for c_ in range(4):
                        nc.tensor.transpose(
                            psD[:, c_:c_ + 1],
                            avs[D:D + 1, c_ * P:(c_ + 1) * P],
                            id128[D:D + 1, D:D + 1])
                    rsb = workp.tile([P, 4], F32, tag="rsb", name="rsb")
                    nc.vector.reciprocal(rsb, psD)
                    psR = psp.tile([1, GW], F32, tag="ps", name="psR")
                    for c_ in range(4):
                        nc.tensor.transpose(
                            psR[0:1, c_ * P:(c_ + 1) * P],
                            rsb[:, c_:c_ + 1], id128)
                    recb = workp.tile([1, GW], BF16, tag="recb", name="recb")
                    nc.vector.tensor_copy(recb, psR)
                    pbc = psp.tile([D, GW], F32, tag="ps", name="pbc")
                    nc.tensor.matmul(pbc, ones4[0:1, :], recb,
                                     start=True, stop=True)
                    at = attn[hh // 2][(hh % 2) * D:(hh % 2 + 1) * D,
                                       ig * GW:(ig + 1) * GW]
                    nc.vector.tensor_tensor(at, avs[0:D, :], pbc,
                                            op=ALU.mult)

            # ---------- E: output projection (transposed) ----------
            # head pairs are stacked along partitions in attn[i]/wores[i],
            # so the c-contraction is two full-128-partition matmuls
            for dt_ in range(8 if phases >= 6 else 0):
                for ng in range(NG):
                    pp = psp.tile([P, GW], F32, tag="ps", name="pp")
                    for i in range(2):
                        nc.tensor.matmul(
                            pp, wores[i][:, dt_ * P:(dt_ + 1) * P],
                            attn[i][:, ng * GW:(ng + 1) * GW],
                            start=(i == 0), stop=(i == 1))
                    ot = outsbp.tile([P, GW], F32, tag="ot", name="ot")
                    if (dt_ * NG + ng) % 2 == 0:
                        nc.scalar.copy(ot, pp)
                    else:
                        nc.vector.tensor_copy(ot, pp)
                    nc.sync.dma_start(
                        out_d[dt_ * P:(dt_ + 1) * P, ng * GW:(ng + 1) * GW],
                        ot)

    nc.compile()
    _NC_CACHE = nc
    return nc


def shard_inputs(x, W_qkv, W_out):
    """Host-side sharding: per-core input dicts (fp16)."""
    x = np.asarray(x, np.float32)
    W_qkv = np.asarray(W_qkv, np.float32)
    W_out = np.asarray(W_out, np.float32)
    scale = D ** -0.5
    W6 = W_qkv.reshape(DIM, 6, H, D)
    in_maps = []
    xT = [np.ascontiguousarray(x[b].T).astype(np.float16) for b in range(2)]
    for c in range(NCORES):
        b, h0 = c // 4, 4 * (c % 4)
        Wc = W6[:, :, h0:h0 + HPC, :].transpose(0, 2, 1, 3).copy()  # [DIM,4,6,D]
        Wc[:, :, 0, :] *= scale  # qu
        Wc[:, :, 3, :] *= scale  # qc
        Wc[:, :, 2, :] *= 0.5    # vu -> Su' = Su/2 for silu-via-tanh
        vc4 = Wc[:, :, 5, :].reshape(DIM, HPC * D)  # vc, natural-layout pass
        # device c-order per head: [qu, vu, ku, qc, vc, kc]
        Wc = Wc[:, :, [0, 2, 1, 3, 5, 4], :]
        wq_c = np.concatenate(
            [Wc.reshape(DIM, HPC * WHEAD), vc4], axis=1).astype(np.float16)
        wo_c = np.ascontiguousarray(
            W_out.reshape(H, D, DIM)[h0:h0 + HPC].reshape(HPC * D, DIM)
        ).astype(np.float16)
        in_maps.append({
            "xT": xT[c // 4],
            "wq": np.ascontiguousarray(wq_c),
            "wo": wo_c,
        })
    return in_maps


def unshard_output(results):
    """results: list of 8 dicts with 'outT' [1024, 2048] partials."""
    outs = []
    for b in range(2):
        acc = results[4 * b]["outT"].astype(np.float32).copy()
        for c in range(4 * b + 1, 4 * b + 4):
            acc += results[c]["outT"]
        outs.append(acc.T)
    return np.stack(outs).astype(np.float32)


def kernel(x, W_qkv, W_out):
    from concourse.bass_utils import run_bass_kernel_spmd

    in_maps = shard_inputs(x, W_qkv, W_out)
    nc = build_nc()
    res = run_bass_kernel_spmd(nc, in_maps, core_ids=list(range(NCORES)))
    return unshard_output(res.results)
